# revision 1
# baseline (speedup 1.0000x reference)
"""Trainium2 Bass kernel for nn_LCAMatrixModel (pairwise selu-MLP scoring).

o[i,j] = hardsigmoid( sum_h W2b[h]*selu(g[i,h]+g[j,h]+b2a[h]) + b2b )
with g = f(x) a small per-node MLP chain. o is symmetric.

Decomposition used on-device (m = min(u,0), e = exp(m), u = g_i+g_j+b2a):
  sum_h w*selu(u) = lam*(c_i + c_j + K0) - lam*sum w*m + lam*al*sum w*e - lam*al*sum w
with c_i = sum_h w[h]*g[i,h] precomputed (rank-1), so only m and e need the
full N^2*H elementwise work. Per pair of output rows (2 i's stacked on 128
partitions as 2x64 h), DVE computes m (one fused add+min), ACT computes e
(exp), and PE reduces over h with zero-padded [128,64] f32r stationary
patterns accumulating into PSUM 64-row groups. Sharding: np.roll(x, -c) per
core -> core c owns global rows {c, c+8, ...}; each core computes only its
local upper triangle (the output is symmetric) and the host mirrors it.
"""
import sys

sys.path.insert(0, "/opt/trn_rl_repo")

import numpy as np

N_NODES = 1536
RAW = 512
D = 128
H = 64
NCORES = 8
ROWS = N_NODES // NCORES * 1  # own rows per core = 192
PAIRS = ROWS // 2             # 96
GROUPS = PAIRS // 32          # 3 groups of 32 pairs (64 out rows each)
NCHUNK = 3                    # 512-wide j chunks

LAM = 1.0507009873554805
AL = 1.6732632423543772

_compiled = None


def _build_program():
    import concourse.bacc as bacc
    import concourse.mybir as mybir
    import concourse.tile as tile

    F32 = mybir.dt.float32
    F32R = mybir.dt.float32r
    AF = mybir.ActivationFunctionType
    OP = mybir.AluOpType

    nc = bacc.Bacc("TRN2", target_bir_lowering=False, debug=False)

    # ---- DRAM I/O ----
    xT_d = nc.dram_tensor("xT", [RAW, N_NODES], F32, kind="ExternalInput")
    wencT_d = nc.dram_tensor("wencT", [RAW, D], F32, kind="ExternalInput")
    benc_d = nc.dram_tensor("benc", [D, 1], F32, kind="ExternalInput")
    w1aT_d = nc.dram_tensor("w1aT", [D, H], F32, kind="ExternalInput")
    b1a_d = nc.dram_tensor("b1a", [H, 1], F32, kind="ExternalInput")
    b1al_d = nc.dram_tensor("b1al", [H, 1], F32, kind="ExternalInput")
    w1bT_d = nc.dram_tensor("w1bT", [H, D], F32, kind="ExternalInput")
    b1b_d = nc.dram_tensor("b1b", [D, 1], F32, kind="ExternalInput")
    b1bl_d = nc.dram_tensor("b1bl", [D, 1], F32, kind="ExternalInput")
    w2aT_d = nc.dram_tensor("w2aT", [D, H], F32, kind="ExternalInput")
    b2a2_d = nc.dram_tensor("b2a2", [D, 1], F32, kind="ExternalInput")
    w32e_d = nc.dram_tensor("w32e", [D, 32 * H], F32R, kind="ExternalInput")
    w32m_d = nc.dram_tensor("w32m", [D, 32 * H], F32R, kind="ExternalInput")
    wl_d = nc.dram_tensor("wl", [H, 1], F32, kind="ExternalInput")
    ones64_d = nc.dram_tensor("ones64", [1, H], F32R, kind="ExternalInput")
    cfin_d = nc.dram_tensor("cfin", [D, 1], F32, kind="ExternalInput")
    out_d = nc.dram_tensor("out", [ROWS, N_NODES], F32, kind="ExternalOutput")

    with tile.TileContext(nc) as tc:
        with (
            tc.tile_pool(name="cst", bufs=1) as cst,
            tc.tile_pool(name="pre", bufs=2) as pre,
            tc.tile_pool(name="mp", bufs=3) as mp,
            tc.tile_pool(name="ep", bufs=3) as ep,
            tc.tile_pool(name="op", bufs=4) as opool,
            tc.tile_pool(name="ps", bufs=6, space="PSUM") as ps,
        ):
            # ---- load constants ----
            xt = cst.tile([D, 4 * N_NODES], F32)
            for k in range(4):
                nc.sync.dma_start(
                    xt[:, k * N_NODES : (k + 1) * N_NODES],
                    xT_d[k * D : (k + 1) * D, :],
                )
            wencT = cst.tile([D, 4 * D], F32)
            for k in range(4):
                nc.sync.dma_start(
                    wencT[:, k * D : (k + 1) * D], wencT_d[k * D : (k + 1) * D, :]
                )
            benc = cst.tile([D, 1], F32)
            nc.sync.dma_start(benc[:], benc_d[:])
            w1aT = cst.tile([D, H], F32)
            nc.sync.dma_start(w1aT[:], w1aT_d[:])
            b1a = cst.tile([H, 1], F32)
            nc.sync.dma_start(b1a[:], b1a_d[:])
            b1al = cst.tile([H, 1], F32)
            nc.sync.dma_start(b1al[:], b1al_d[:])
            w1bT = cst.tile([H, D], F32)
            nc.sync.dma_start(w1bT[:], w1bT_d[:])
            b1b = cst.tile([D, 1], F32)
            nc.sync.dma_start(b1b[:], b1b_d[:])
            b1bl = cst.tile([D, 1], F32)
            nc.sync.dma_start(b1bl[:], b1bl_d[:])
            w2aT = cst.tile([D, H], F32)
            nc.sync.dma_start(w2aT[:], w2aT_d[:])
            b2a2 = cst.tile([D, 1], F32)
            nc.sync.dma_start(b2a2[:], b2a2_d[:])
            w32e = cst.tile([D, 32 * H], F32R)
            nc.sync.dma_start(w32e[:], w32e_d[:])
            w32m = cst.tile([D, 32 * H], F32R)
            nc.sync.dma_start(w32m[:], w32m_d[:])
            wl = cst.tile([H, 1], F32)
            nc.sync.dma_start(wl[:], wl_d[:])
            ones64 = cst.tile([1, H], F32R)
            nc.sync.dma_start(ones64[:], ones64_d[:])
            cfin = cst.tile([D, 1], F32)
            nc.sync.dma_start(cfin[:], cfin_d[:])

            CW = 512  # chunk width

            # ---- zT = W_encT.T @ xT + b_enc  [128, 1536] ----
            zT = cst.tile([D, N_NODES], F32)
            for c in range(NCHUNK):
                pz = ps.tile([D, CW], F32, tag="ps")
                for k in range(4):
                    nc.tensor.matmul(
                        pz[:],
                        wencT[:, k * D : (k + 1) * D],
                        xt[:, k * N_NODES + c * CW : k * N_NODES + (c + 1) * CW],
                        start=(k == 0),
                        stop=(k == 3),
                    )
                nc.scalar.activation(
                    zT[:, c * CW : (c + 1) * CW], pz[:], AF.Identity, bias=benc[:, 0:1]
                )

            # ---- selu chain helper: out_sb chunk = selu(psum + b) ----
            def selu_from_psum(out_ap, pa, b_raw, b_lam, p):
                # r = lam*relu(v+b) = relu(lam*v + lam*b)
                r = pre.tile([p, CW], F32, tag="selr")
                nc.scalar.activation(r[:], pa, AF.Relu, bias=b_lam, scale=LAM)
                # m = min(v+b, 0); t = lam*al*(exp(m)-1)
                m = pre.tile([p, CW], F32, tag="selm")
                nc.vector.tensor_scalar(m[:], pa, b_raw, 0.0, OP.add, OP.min)
                e = pre.tile([p, CW], F32, tag="sele")
                nc.scalar.activation(e[:], m[:], AF.Exp)
                t = pre.tile([p, CW], F32, tag="selt")
                nc.vector.tensor_scalar(t[:], e[:], LAM * AL, -LAM * AL, OP.mult, OP.add)
                nc.vector.tensor_tensor(out_ap, r[:], t[:], OP.add)

            # ---- a1T = selu(W1aT.T @ zT + b1a)  [64, 1536] ----
            a1T = cst.tile([H, N_NODES], F32)
            for c in range(NCHUNK):
                pa = ps.tile([H, CW], F32, tag="ps")
                nc.tensor.matmul(
                    pa[:], w1aT[:], zT[:, c * CW : (c + 1) * CW], start=True, stop=True
                )
                selu_from_psum(
                    a1T[:, c * CW : (c + 1) * CW], pa[:], b1a[:, 0:1], b1al[:, 0:1], H
                )

            # ---- hT = selu(W1bT.T @ a1T + b1b)  [128, 1536] ----
            hT = cst.tile([D, N_NODES], F32)
            for c in range(NCHUNK):
                ph = ps.tile([D, CW], F32, tag="ps")
                nc.tensor.matmul(
                    ph[:], w1bT[:], a1T[:, c * CW : (c + 1) * CW], start=True, stop=True
                )
                selu_from_psum(
                    hT[:, c * CW : (c + 1) * CW], ph[:], b1b[:, 0:1], b1bl[:, 0:1], D
                )

            # ---- g2 = [gT; gT]  [128, 1536], gT = W2aT.T @ hT ----
            g2 = cst.tile([D, N_NODES], F32)
            for c in range(NCHUNK):
                pg = ps.tile([H, CW], F32, tag="ps")
                nc.tensor.matmul(
                    pg[:], w2aT[:], hT[:, c * CW : (c + 1) * CW],
                    start=True, stop=True,
                )
                nc.scalar.activation(g2[0:H, c * CW : (c + 1) * CW], pg[:], AF.Copy)
            nc.sync.dma_start(g2[H:D, :], g2[0:H, :])

            # ---- gbs[:, t] = [gT[:,16t]+b2a ; gT[:,16t+8]+b2a]  [128, 96] ----
            gbs_raw = cst.tile([D, PAIRS], F32)
            g2_top = g2[0:H, :].rearrange("p (a b) -> p a b", b=16)
            g2_bot = g2[H:D, :].rearrange("p (a b) -> p a b", b=16)
            nc.sync.dma_start(gbs_raw[0:H, :], g2_top[:, :, 0:1])
            nc.sync.dma_start(gbs_raw[H:D, :], g2_bot[:, :, 8:9])
            gbs = cst.tile([D, PAIRS], F32)
            nc.vector.tensor_scalar_add(gbs[:], gbs_raw[:], b2a2[:, 0:1])

            # ---- gown [64, 192]: own-node columns of gT (stride 8) ----
            gown = cst.tile([H, ROWS], F32)
            g2_own = g2[0:H, :].rearrange("p (a b) -> p a b", b=8)
            nc.sync.dma_start(gown[:], g2_own[:, :, 0:1])

            # ---- c_row [1, 1536] = wl.T @ gT  (includes lam) ----
            c_row = cst.tile([1, N_NODES], F32R)
            for c in range(NCHUNK):
                pc = ps.tile([1, CW], F32, tag="ps")
                nc.tensor.matmul(
                    pc[:], wl[:], g2[0:H, c * CW : (c + 1) * CW], start=True, stop=True
                )
                nc.scalar.activation(c_row[0:1, c * CW : (c + 1) * CW], pc[:], AF.Copy)

            # ---- Bcol [64, 3]: (c_i + CONST)/6 + 0.5, one column per group ----
            Bcol = cst.tile([H, GROUPS], F32)
            for G in range(GROUPS):
                pb = ps.tile([H, 1], F32, tag="ps", name=f"pb_{G}")
                nc.tensor.matmul(
                    pb[:], gown[:, H * G : H * (G + 1)], wl[:], start=True, stop=True
                )
                nc.scalar.activation(
                    Bcol[:, G : G + 1], pb[:], AF.Identity,
                    scale=1.0 / 6.0, bias=cfin[0:H, 0:1],
                )

            # ---- main pairwise loop (triangular) ----
            # group G (32 pairs) -> out rows [64G, 64G+64), needs chunks c >= G
            # each (G, c) gets its own [64, CW] psum tile at base partition 0
            # (f32r matmuls cannot target base partition 64)
            def finalize(psum_t, bcol_ap, c, row0):
                o = opool.tile([H, CW], F32, tag="o", name=f"o_{row0}_{c}")
                nc.scalar.activation(
                    o[:], psum_t[:], AF.Relu, scale=1.0 / 6.0, bias=bcol_ap
                )
                nc.vector.tensor_scalar_min(o[:], o[:], 1.0)
                nc.sync.dma_start(
                    out_d[row0 : row0 + 64, c * CW : (c + 1) * CW], o[:]
                )

            for G in range(GROUPS):
                W = N_NODES - CW * G
                psum_grp = {
                    c: ps.tile([H, CW], F32, tag="ps", name=f"psg_{G}_{c}")
                    for c in range(G, NCHUNK)
                }
                for u in range(16):  # pair batches of 2
                    m2 = mp.tile([D, 2 * N_NODES], F32R, tag="m2")
                    e2 = ep.tile([D, 2 * N_NODES], F32R, tag="e2")
                    for q in range(2):
                        t = 32 * G + 2 * u + q
                        nc.vector.tensor_scalar(
                            m2[:, q * W : (q + 1) * W],
                            g2[:, CW * G : N_NODES],
                            gbs[:, t : t + 1],
                            0.0,
                            OP.add,
                            OP.min,
                        )
                    nc.scalar.activation(
                        e2[:, 0 : 2 * W], m2[:, 0 : 2 * W].bitcast(F32), AF.Exp
                    )
                    for q in range(2):
                        t = 32 * G + 2 * u + q
                        s = t % 32
                        for c in range(G, NCHUNK):
                            pt = psum_grp[c]
                            j0 = q * W + (c - G) * CW
                            nc.tensor.matmul(
                                pt[:],
                                w32e[:, H * s : H * (s + 1)],
                                e2[:, j0 : j0 + CW],
                                start=(s == 0),
                                stop=False,
                                skip_group_check=True,
                            )
                            nc.tensor.matmul(
                                pt[:],
                                w32m[:, H * s : H * (s + 1)],
                                m2[:, j0 : j0 + CW],
                                start=False,
                                stop=False,
                                skip_group_check=True,
                            )
                # rank-1 c_j add closes each (G, chunk); then finalize
                bcol_ap = Bcol[:, G : G + 1]
                for c in range(G, NCHUNK):
                    pt = psum_grp[c]
                    nc.tensor.matmul(
                        pt[:],
                        ones64[:],
                        c_row[0:1, c * CW : (c + 1) * CW],
                        start=False,
                        stop=True,
                        skip_group_check=True,
                    )
                    finalize(pt, bcol_ap, c, 64 * G)

    nc.compile()
    return nc


def _host_inputs(x, W_enc, b_enc, W1a, b1a, W1b, b1b, W2a, b2a, W2b, b2b):
    """Build the per-core input maps (core c gets x rolled by -c)."""
    w = W2b[0].astype(np.float64)
    K0 = float(w @ b2a.astype(np.float64))
    SW = float(w.sum())
    CONST = LAM * K0 - LAM * AL * SW + float(b2b[0])

    w32e = np.zeros((D, 32 * H), np.float32)
    w32m = np.zeros((D, 32 * H), np.float32)
    for s in range(32):
        w32e[0:H, s * H + 2 * s] = (LAM * AL * w).astype(np.float32)
        w32e[H:D, s * H + 2 * s + 1] = (LAM * AL * w).astype(np.float32)
        w32m[0:H, s * H + 2 * s] = (-LAM * w).astype(np.float32)
        w32m[H:D, s * H + 2 * s + 1] = (-LAM * w).astype(np.float32)

    common = {
        "wencT": np.ascontiguousarray(W_enc.T, np.float32),
        "benc": b_enc.reshape(D, 1).astype(np.float32),
        "w1aT": np.ascontiguousarray(W1a.T, np.float32),
        "b1a": b1a.reshape(H, 1).astype(np.float32),
        "b1al": (LAM * b1a).reshape(H, 1).astype(np.float32),
        "w1bT": np.ascontiguousarray(W1b.T, np.float32),
        "b1b": b1b.reshape(D, 1).astype(np.float32),
        "b1bl": (LAM * b1b).reshape(D, 1).astype(np.float32),
        "w2aT": np.ascontiguousarray(W2a.T, np.float32),
        "b2a2": np.concatenate([b2a, b2a]).reshape(D, 1).astype(np.float32),
        "w32e": w32e,
        "w32m": w32m,
        "wl": (LAM * w).reshape(H, 1).astype(np.float32),
        "ones64": np.ones((1, H), np.float32),
        "cfin": np.full((D, 1), CONST / 6.0 + 0.5, np.float32),
    }
    in_maps = []
    for c in range(NCORES):
        m = dict(common)
        m["xT"] = np.ascontiguousarray(np.roll(x, -c, axis=0).T, np.float32)
        in_maps.append(m)
    return in_maps


def _assemble(results):
    """Mirror per-core upper-triangle bands into the full symmetric output."""
    O = np.zeros((N_NODES, N_NODES), np.float32)
    for c in range(NCORES):
        U = np.roll(results[c]["out"], c, axis=1)  # undo column roll
        O[c::8, :] = U  # rows c, c+8, ... (192 rows in order)
    Ou = np.triu(O)
    return (Ou + Ou.T - np.diag(np.diag(Ou))).astype(np.float32)


def kernel(x, W_enc, b_enc, W1a, b1a, W1b, b1b, W2a, b2a, W2b, b2b):
    from concourse.bass_utils import run_bass_kernel_spmd

    global _compiled
    if _compiled is None:
        _compiled = _build_program()
    in_maps = _host_inputs(
        np.asarray(x, np.float32),
        np.asarray(W_enc, np.float32), np.asarray(b_enc, np.float32),
        np.asarray(W1a, np.float32), np.asarray(b1a, np.float32),
        np.asarray(W1b, np.float32), np.asarray(b1b, np.float32),
        np.asarray(W2a, np.float32), np.asarray(b2a, np.float32),
        np.asarray(W2b, np.float32), np.asarray(b2b, np.float32),
    )
    res = run_bass_kernel_spmd(_compiled, in_maps, list(range(NCORES)))
    return _assemble(res.results)

